# revision 64
# baseline (speedup 1.0000x reference)
"""Causal multi-head attention (B=2, S=2048, D=1024, H=16) on 8 TRN2 NeuronCores.

Sharding: core c -> batch b = c//4, head-group g = c%4 (4 heads = 256 e-dims).
Each core computes its heads' attention plus the row-parallel slice of the out
projection; the host sums the 4 partial outputs per batch.

Matmul operand dtype is switchable (MM_DTYPE): bfloat16 (full PE rate) or
float32r (TF32-like, ~half rate, 8x less rounding error). Host pre-rounds
operands bitwise so device rounding is the identity. Scores are computed
transposed (S.T[sk, sq]) so softmax needs no reductions: exp without
max-subtraction (|scaled scores| < ~4), causal handled by a triangular mask
multiply after exp. V carries 64 extra all-ones columns so the PV matmul
produces both ctx.T (rows 0:64) and the softmax denominator broadcast to rows
64:128 for free; normalization is then reciprocal + multiply on DVE.
"""
import numpy as np
import ml_dtypes

import concourse.bass as bass
import concourse.tile as tile
from concourse import mybir
from concourse.bass import ts, ds
from concourse.bass_utils import run_bass_kernel_spmd

B, S, D, H = 2, 2048, 1024, 16
DH = D // H
P = 128
HC = 4            # heads per core
E = HC * DH       # 256 e-dims per core
ET = E // P       # 2 e-tiles
DT = D // P       # 8 d-tiles
NST = S // P      # 16 s-tiles
QC = 512          # attention sq-chunk
NQC = S // QC     # 4 attention chunks


def _sch():
    """Projection s-chunk width: 512 for bf16; 256 for f32r (SBUF budget)."""
    return 512 if MM_DTYPE == "bf16" else 256

f32 = mybir.dt.float32
AF = mybir.ActivationFunctionType
OP = mybir.AluOpType

MM_DTYPE = "bf16"   # "bf16" | "f32r"


def _mmdt():
    return mybir.dt.bfloat16 if MM_DTYPE == "bf16" else mybir.dt.float32r


def _np_round(x: np.ndarray) -> np.ndarray:
    """Host-side rounding matching the device matmul operand dtype."""
    x = np.ascontiguousarray(x, dtype=np.float32)
    if MM_DTYPE == "bf16":
        return x.astype(ml_dtypes.bfloat16)
    u = x.view(np.uint32).astype(np.uint64)
    u = u + 0x7FF + ((u >> 12) & 1)
    return (u & 0xFFFFF000).astype(np.uint32).view(np.float32)


def _zero(nc, ap):
    it = mybir.dt.uint16 if MM_DTYPE == "bf16" else mybir.dt.uint32
    nc.vector.memset(ap.bitcast(it), 0)


def _one_fill(nc, ap):
    if MM_DTYPE == "bf16":
        nc.vector.memset(ap.bitcast(mybir.dt.uint16), 0x3F80)
    else:
        nc.vector.memset(ap.bitcast(mybir.dt.uint32), 0x3F800000)


def legalize_multi_waits(nc) -> int:
    """Walrus STRUCT-decoded instructions accept a single sem wait; hoist extra
    waits onto preceding same-engine NoOps (same-engine program order preserves
    semantics exactly)."""
    n = 0
    for fn in nc.m.functions:
        for block in fn.blocks:
            new_insts = []
            for inst in block.instructions:
                si = getattr(inst, "sync_info", None)
                if (si is not None and si.on_wait and len(si.on_wait) > 1
                        and not isinstance(inst, mybir.InstNoOp)):
                    waits = list(si.on_wait)
                    for i, w in enumerate(waits[:-1]):
                        new_insts.append(mybir.InstNoOp(
                            name=f"{inst.name}-wsplit{i}", engine=inst.engine,
                            bass_nofuse=True,
                            sync_info=mybir.SyncInfo(on_wait=[w], on_update=[])))
                        n += 1
                    inst.sync_info = mybir.SyncInfo(
                        on_wait=[waits[-1]], on_update=si.on_update)
                new_insts.append(inst)
            block.instructions[:] = new_insts
    return n


def _attention_chunk_pair(nc, C, hpair, khT_sb, qhT_sb, vh_sb, tri_sb, pt_pool,
                          st_pool, cx_pool, rec_pool, ctxT_sb):
    """Attention for sq-chunk C for head pair (2*hpair, 2*hpair+1). The two
    heads' K=64 S.T matmuls are emitted back-to-back targeting PE row groups
    0-63 / 64-127 so they run concurrently in the array."""
    mdt = _mmdt()
    heads = (2 * hpair, 2 * hpair + 1)
    cxs = {}
    for h in heads:
        cx_h = cx_pool.tile([P, QC], f32, tag="cx", name=f"cx{h}")
        cxs[h] = cx_h
    jmax = 4 * C + 3

    def st_stage(j):
        off = max(0, (j - 4 * C) * P)      # masked prefix width inside chunk
        lo = C * QC + off
        w = QC - off
        # pair-fused S.T psum / P.T tiles: head h at column block 512*(h%2)
        stp = st_pool.tile([P, 2 * QC], f32, tag="st")
        pt = pt_pool.tile([P, 2 * QC], mdt, tag="pt")
        for h in heads:
            pb = DH * (h % 2)
            kh = khT_sb[pb:pb + DH, h // 2, :]
            qh = qhT_sb[pb:pb + DH, h // 2, :]
            # S.T tile: [sk=128 of tile j, sq=w]; the two heads target PE row
            # groups 0-63 / 64-127 and run concurrently in the array.
            nc.tensor.matmul(stp[:, ds(QC * (h % 2), w)], kh[:, ts(j, P)],
                             qh[:, ds(lo, w)], start=True, stop=True)
        # one wide exp for both heads: P.T = exp(S.T / 8), PSUM -> SBUF
        pt_r = pt.rearrange("p (h q) -> p h q", h=2)
        stp_r = stp.rearrange("p (h q) -> p h q", h=2)
        nc.scalar.activation(
            out=pt_r[:, :, off:QC], in_=stp_r[:, :, 0:w],
            func=AF.Exp, scale=float(DH) ** -0.5)
        if j >= 4 * C:
            # diagonal tile: zero future positions (sq < sk); one fused op
            # over both heads with tri broadcast along the pair dim
            tri_bc = tri_sb.rearrange("p (o f) -> p o f", o=1).to_broadcast(
                [P, 2, P])
            nc.vector.tensor_tensor(
                out=pt_r[:, :, off:off + P], in0=pt_r[:, :, off:off + P],
                in1=tri_bc, op=OP.mult)
        return pt

    def pv_stage(j, pt):
        off = max(0, (j - 4 * C) * P)
        w = QC - off
        for h in heads:
            # PV accumulate: rows 0:64 ctx.T, rows 64:128 l (V ones-block).
            # The masked prefix [0, off) is skipped: those sq positions get
            # no contribution from sk-tile j.
            nc.tensor.matmul(cxs[h][:, off:QC],
                             vh_sb[:, j, h, :],
                             pt[:, ds(QC * (h % 2) + off, w)],
                             start=(j == 0), stop=(j == jmax),
                             skip_group_check=True)

    # 1-step software skew: each PV trails its S.T by one j so a PV stalled
    # on a cx slot always has the next S.T already issued on the PE
    pts = {}
    SKEW = 3
    for j in range(jmax + 1):
        pts[j] = st_stage(j)
        if j >= SKEW:
            pv_stage(j - SKEW, pts.pop(j - SKEW))
    for j in range(max(0, jmax - SKEW + 1), jmax + 1):
        pv_stage(j, pts.pop(j))
    for h in heads:
        # normalize: 1/l = exp(-ln(l)) on ACT (DVE reciprocal is ~6 cyc/elem
        # and its latency would sit on the PSUM-release chain); l is broadcast
        # in rows 64:128 by the V ones-block.
        cx = cxs[h]
        lt = rec_pool.tile([DH, QC], f32, tag="lt")
        nc.scalar.activation(out=lt[:, :], in_=cx[DH:2 * DH, :], func=AF.Ln)
        rinv = rec_pool.tile([DH, QC], f32, tag="rinv")
        nc.scalar.activation(out=rinv[:, :], in_=lt[:, :], func=AF.Exp,
                             scale=-1.0)
        pbase, pair = DH * (h % 2), h // 2
        nc.vector.tensor_tensor(
            out=ctxT_sb[pbase:pbase + DH, pair, ds(C * QC, QC)],
            in0=cx[0:DH, :], in1=rinv[:, :], op=OP.mult)


def build():
    mdt = _mmdt()
    SCH = _sch()
    NCH = S // SCH
    nc = bass.Bass("TRN2", enable_partition_id=False)

    qt = nc.dram_tensor("qt", [NCH, P, DT, SCH], mdt, kind="ExternalInput")
    kt = nc.dram_tensor("kt", [NCH, P, DT, SCH], mdt, kind="ExternalInput")
    vt = nc.dram_tensor("vt", [NCH, P, DT, SCH], mdt, kind="ExternalInput")
    wq = nc.dram_tensor("wq", [P, DT, E], mdt, kind="ExternalInput")
    wk = nc.dram_tensor("wk", [P, DT, E], mdt, kind="ExternalInput")
    wv = nc.dram_tensor("wv", [P, DT, E], mdt, kind="ExternalInput")
    wp = nc.dram_tensor("wp", [P, 2, D], mdt, kind="ExternalInput")
    bq2 = nc.dram_tensor("bq2", [P, ET], f32, kind="ExternalInput")
    bk2 = nc.dram_tensor("bk2", [P, ET], f32, kind="ExternalInput")
    bvr = nc.dram_tensor("bvr", [1, E], mdt, kind="ExternalInput")
    onesr = nc.dram_tensor("onesr", [1, E], mdt, kind="ExternalInput")
    tri = nc.dram_tensor("tri", [P, P], mdt, kind="ExternalInput")
    bpr = nc.dram_tensor("bpr", [1, D], mdt, kind="ExternalInput")
    outp = nc.dram_tensor("outp", [S, D], f32, kind="ExternalOutput")

    with tile.TileContext(nc) as tc:
        with (
            tc.tile_pool(name="consts", bufs=1) as consts,
            tc.tile_pool(name="inp", bufs=2) as inp,
            tc.tile_pool(name="acts", bufs=1) as acts,
            tc.tile_pool(name="pt", bufs=6) as pt_pool,
            tc.tile_pool(name="rec", bufs=2) as rec_pool,
            tc.tile_pool(name="osb", bufs=2) as osb_pool,
            tc.tile_pool(name="projps", bufs=2, space="PSUM") as proj_ps,
            tc.tile_pool(name="stps", bufs=2, space="PSUM") as st_pool,
            tc.tile_pool(name="cxps", bufs=2, space="PSUM") as cx_pool,
        ):
            out_ps = cx_pool  # out-proj psums share the cx slots (same shape)
            # ---- constants ----
            wq_sb = consts.tile([P, DT, E], mdt)
            wk_sb = consts.tile([P, DT, E], mdt)
            wv_sb = consts.tile([P, DT, E], mdt)
            wp_sb = consts.tile([P, 2, D], mdt)
            bq_sb = consts.tile([P, ET], f32)
            bk_sb = consts.tile([P, ET], f32)
            bvr_sb = consts.tile([1, E], mdt)
            ones_sb = consts.tile([1, E], mdt)
            tri_sb = consts.tile([P, P], mdt)
            bp_sb = consts.tile([P, D], mdt)


            # PE warmup: dummy matmuls during the startup DMA window release
            # the HAM clock throttle before the first real matmul arrives
            warm = consts.tile([P, QC], mdt)
            nc.vector.memset(warm[:].bitcast(
                mybir.dt.uint16 if MM_DTYPE == "bf16" else mybir.dt.uint32), 0)
            wps = proj_ps.tile([P, SCH], f32, tag="proj")
            for _ in range(26):
                nc.tensor.matmul(wps[:, :], warm[:, 0:P], warm[:, :],
                                 start=True, stop=True)

            # ---- persistent activations ----
            qhT_sb = acts.tile([P, ET, S], mdt)    # Qh.T  (e on partitions)
            khT_sb = acts.tile([P, ET, S], mdt)    # Kh.T
            vh_sb = acts.tile([P, NST, HC, 2 * DH], mdt)  # [V | ones-block]
            ctxT_sb = acts.tile([P, 2, S], mdt)    # normalized ctx.T head pairs

            # ones block of vh (cols 64:128) - broadcasts l in the PV matmul
            _one_fill(nc, vh_sb[:, :, :, DH:2 * DH])

            # attention/out-proj constants: needed later, DMA'd after chunk 0
            def late_const_dmas():
                nc.sync.dma_start(out=tri_sb[:], in_=tri[:, :])
                nc.sync.dma_start(out=wp_sb[:], in_=wp[:, :, :])
                nc.sync.dma_start(out=bp_sb[:],
                                  in_=bpr[0:1, :].to_broadcast([P, D]))

            helpers = {}
            for ch in range(NCH):
                qc = inp.tile([P, DT, SCH], mdt, tag="qc")
                kc = inp.tile([P, DT, SCH], mdt, tag="kc")
                vc = inp.tile([P, DT, SCH], mdt, tag="vc")
                if ch == 0:
                    # SP issues DMA triggers in order: sequence so the first
                    # Q-proj matmuls' deps (wq, bq, first half of qc) land
                    # first, then K's, then the rest.
                    hd = DT // 2
                    nc.sync.dma_start(out=wq_sb[:], in_=wq[:, :, :])
                    nc.sync.dma_start(out=bq_sb[:], in_=bq2[:, :])
                    nc.sync.dma_start(out=qc[:, 0:hd, :], in_=qt[ch, :, 0:hd, :])
                    nc.sync.dma_start(out=wk_sb[:], in_=wk[:, :, :])
                    nc.sync.dma_start(out=bk_sb[:], in_=bk2[:, :])
                    nc.sync.dma_start(out=kc[:, 0:hd, :], in_=kt[ch, :, 0:hd, :])
                    nc.sync.dma_start(out=qc[:, hd:DT, :],
                                      in_=qt[ch, :, hd:DT, :])
                    nc.sync.dma_start(out=kc[:, hd:DT, :],
                                      in_=kt[ch, :, hd:DT, :])
                    nc.sync.dma_start(out=wv_sb[:], in_=wv[:, :, :])
                    nc.sync.dma_start(out=bvr_sb[:], in_=bvr[:, :])
                    nc.sync.dma_start(out=ones_sb[:], in_=onesr[:, :])
                    nc.sync.dma_start(out=vc[:], in_=vt[ch])
                    late_const_dmas()
                else:
                    nc.sync.dma_start(out=qc[:], in_=qt[ch])
                    nc.sync.dma_start(out=kc[:], in_=kt[ch])
                    nc.sync.dma_start(out=vc[:], in_=vt[ch])

                # Q / K projections -> [e, s] layout
                for (src, w_sb, b_sb, dst) in (
                    (qc, wq_sb, bq_sb, qhT_sb),
                    (kc, wk_sb, bk_sb, khT_sb),
                ):
                    for et in range(ET):
                        pp = proj_ps.tile([P, SCH], f32, tag="proj")
                        for d in range(DT):
                            nc.tensor.matmul(pp[:, :], w_sb[:, d, ts(et, P)],
                                             src[:, d, :],
                                             start=(d == 0), stop=(d == DT - 1))
                        nc.vector.tensor_scalar_add(
                            out=dst[:, et, ts(ch, SCH)], in0=pp[:, :],
                            scalar1=b_sb[:, et:et + 1])

                # V projection -> [s, e] natural layout (+ bias via ones row)
                for st2 in range(SCH // P):
                    s_tile = ch * (SCH // P) + st2
                    pp = proj_ps.tile([P, SCH], f32, tag="proj")
                    for d in range(DT):
                        nc.tensor.matmul(pp[:, 0:E], vc[:, d, ds(st2 * P, P)],
                                         wv_sb[:, d, :],
                                         start=(d == 0), stop=False)
                    nc.tensor.matmul(pp[:, 0:E], ones_sb[0:1, 0:P],
                                     bvr_sb[0:1, :], start=False, stop=True)
                    # one fused strided copy scatters all 4 heads' V slices
                    nc.vector.tensor_copy(
                        out=vh_sb[:, s_tile, :, 0:DH],
                        in_=pp[:, 0:E].rearrange("p (h d) -> p h d", h=HC))

                def outproj(C):
                    split_dma = (C == NQC - 1)  # drain the tail sooner
                    for t in range(4 * C, 4 * C + 4):
                        o_sb = osb_pool.tile([P, D], f32, tag="osb")
                        for nch2 in range(2):
                            op = out_ps.tile([P, QC], f32, tag="cx")
                            for pair in range(2):
                                nc.tensor.matmul(
                                    op[:, :], ctxT_sb[:, pair, ts(t, P)],
                                    wp_sb[:, pair, ds(nch2 * QC, QC)],
                                    start=(pair == 0), stop=(pair == 1))
                            nc.vector.tensor_tensor(
                                out=o_sb[:, ds(nch2 * QC, QC)], in0=op[:, :],
                                in1=bp_sb[:, ds(nch2 * QC, QC)], op=OP.add)
                            if split_dma:
                                nc.sync.dma_start(
                                    out=outp[ts(t, P), ds(nch2 * QC, QC)],
                                    in_=o_sb[:, ds(nch2 * QC, QC)])
                        if not split_dma:
                            nc.sync.dma_start(out=outp[ts(t, P), :],
                                              in_=o_sb[:])

                def attn(C):
                    for hpair in range(HC // 2):
                        _attention_chunk_pair(nc, C, hpair, khT_sb, qhT_sb,
                                              vh_sb, tri_sb, pt_pool, st_pool,
                                              cx_pool, rec_pool, ctxT_sb)

                helpers["attn"], helpers["outproj"] = attn, outproj

                if SCH == QC:
                    # shifted schedule: proj(C+1) is already emitted before
                    # attention(C), so chunk boundaries only carry the small
                    # out-proj on the PE instead of proj+out-proj
                    if ch >= 1:
                        C = ch - 1
                        if C > 0:
                            outproj(C - 1)
                        attn(C)
                elif ((ch + 1) * SCH) % QC == 0:
                    C = ((ch + 1) * SCH) // QC - 1
                    if C > 0:
                        outproj(C - 1)
                    attn(C)
                    if C == NQC - 1:
                        outproj(C)
            if SCH == QC:
                helpers["outproj"](NQC - 2)
                helpers["attn"](NQC - 1)
                helpers["outproj"](NQC - 1)
    legalize_multi_waits(nc)
    return nc


_NC_CACHE = {}


def _shard(inputs):
    q, k, v = inputs["q"], inputs["k"], inputs["v"]
    Wq, bq, Wk, bk = inputs["Wq"], inputs["bq"], inputs["Wk"], inputs["bk"]
    Wv, bv, Wp, bp = inputs["Wv"], inputs["bv"], inputs["Wp"], inputs["bp"]

    SCH = _sch()
    NCH = S // SCH

    def xT_layout(x):       # [S, D] -> [NCH, P, DT, SCH]
        a = _np_round(np.ascontiguousarray(np.asarray(x).T))      # [D, S]
        return np.ascontiguousarray(
            a.reshape(DT, P, NCH, SCH).transpose(2, 1, 0, 3))

    def w_layout(W, g):     # -> [P, DT, E]   (W[gdims,:].T tiled)
        a = _np_round(np.ascontiguousarray(np.asarray(W)[g * E:(g + 1) * E, :].T))
        return np.ascontiguousarray(a.reshape(DT, P, E).transpose(1, 0, 2))

    def wp_layout(Wp_, g):  # -> [P, 2, D]    (Wp[:, gdims].T tiled)
        a = _np_round(np.ascontiguousarray(np.asarray(Wp_)[:, g * E:(g + 1) * E].T))
        return np.ascontiguousarray(a.reshape(2, P, D).transpose(1, 0, 2))

    tri_np = _np_round(np.triu(np.ones((P, P), np.float32)))  # keep sq >= sk
    ones_np = _np_round(np.ones((1, E), np.float32))
    zeros_bp = np.zeros(D, np.float32)

    in_maps = []
    for c in range(8):
        b, g = c // 4, c % 4
        m = {
            "qt": xT_layout(q[b]),
            "kt": xT_layout(k[b]),
            "vt": xT_layout(v[b]),
            "wq": w_layout(Wq, g),
            "wk": w_layout(Wk, g),
            "wv": w_layout(Wv, g),
            "wp": wp_layout(Wp, g),
            "bq2": np.ascontiguousarray(
                np.asarray(bq, np.float32)[g * E:(g + 1) * E].reshape(ET, P).T),
            "bk2": np.ascontiguousarray(
                np.asarray(bk, np.float32)[g * E:(g + 1) * E].reshape(ET, P).T),
            "bvr": _np_round(np.asarray(bv)[g * E:(g + 1) * E].reshape(1, E)),
            "onesr": ones_np,
            "tri": tri_np,
            "bpr": _np_round(np.asarray(bp, np.float32).reshape(1, D))
                   if g == 0 else _np_round(zeros_bp.reshape(1, D)),
        }
        in_maps.append(m)
    return in_maps


def run(inputs, trace: bool = False, trace_cores=None):
    if MM_DTYPE not in _NC_CACHE:
        _NC_CACHE[MM_DTYPE] = build()
    nc = _NC_CACHE[MM_DTYPE]
    in_maps = _shard(inputs)
    kw = {}
    if trace:
        kw = dict(trace=True,
                  trace_cores=trace_cores if trace_cores is not None else [0])
    res = run_bass_kernel_spmd(nc, in_maps, core_ids=list(range(8)), **kw)
    out = np.zeros((B, S, D), np.float32)
    for c in range(8):
        out[c // 4] += res.results[c]["outp"]
    return out, res


def kernel(**inputs) -> np.ndarray:
    return run(inputs)[0]


# revision 66
# speedup vs baseline: 1.1168x; 1.1168x over previous
"""Causal multi-head attention (B=2, S=2048, D=1024, H=16) on 8 TRN2 NeuronCores.

Sharding: core c -> batch b = c//4, head-group g = c%4 (4 heads = 256 e-dims).
Each core computes its heads' attention plus the row-parallel slice of the out
projection; the host sums the 4 partial outputs per batch.

Matmul operand dtype is switchable (MM_DTYPE): bfloat16 (full PE rate) or
float32r (TF32-like, ~half rate, 8x less rounding error). Host pre-rounds
operands bitwise so device rounding is the identity. Scores are computed
transposed (S.T[sk, sq]) so softmax needs no reductions: exp without
max-subtraction (|scaled scores| < ~4), causal handled by a triangular mask
multiply after exp. V carries 64 extra all-ones columns so the PV matmul
produces both ctx.T (rows 0:64) and the softmax denominator broadcast to rows
64:128 for free; normalization is then reciprocal + multiply on DVE.
"""
import numpy as np
import ml_dtypes

import concourse.bass as bass
import concourse.tile as tile
from concourse import mybir
from concourse.bass import ts, ds
from concourse.bass_utils import run_bass_kernel_spmd

B, S, D, H = 2, 2048, 1024, 16
DH = D // H
P = 128
HC = 4            # heads per core
E = HC * DH       # 256 e-dims per core
ET = E // P       # 2 e-tiles
DT = D // P       # 8 d-tiles
NST = S // P      # 16 s-tiles
QC = 512          # attention sq-chunk
NQC = S // QC     # 4 attention chunks


def _sch():
    """Projection s-chunk width: 512 for bf16; 256 for f32r (SBUF budget)."""
    return 512 if MM_DTYPE == "bf16" else 256

f32 = mybir.dt.float32
AF = mybir.ActivationFunctionType
OP = mybir.AluOpType

MM_DTYPE = "bf16"   # "bf16" | "f32r"


def _mmdt():
    return mybir.dt.bfloat16 if MM_DTYPE == "bf16" else mybir.dt.float32r


def _np_round(x: np.ndarray) -> np.ndarray:
    """Host-side rounding matching the device matmul operand dtype."""
    x = np.ascontiguousarray(x, dtype=np.float32)
    if MM_DTYPE == "bf16":
        return x.astype(ml_dtypes.bfloat16)
    u = x.view(np.uint32).astype(np.uint64)
    u = u + 0x7FF + ((u >> 12) & 1)
    return (u & 0xFFFFF000).astype(np.uint32).view(np.float32)


def _zero(nc, ap):
    it = mybir.dt.uint16 if MM_DTYPE == "bf16" else mybir.dt.uint32
    nc.vector.memset(ap.bitcast(it), 0)


def _one_fill(nc, ap):
    if MM_DTYPE == "bf16":
        nc.vector.memset(ap.bitcast(mybir.dt.uint16), 0x3F80)
    else:
        nc.vector.memset(ap.bitcast(mybir.dt.uint32), 0x3F800000)


def legalize_multi_waits(nc) -> int:
    """Walrus STRUCT-decoded instructions accept a single sem wait; hoist extra
    waits onto preceding same-engine NoOps (same-engine program order preserves
    semantics exactly)."""
    n = 0
    for fn in nc.m.functions:
        for block in fn.blocks:
            new_insts = []
            for inst in block.instructions:
                si = getattr(inst, "sync_info", None)
                if (si is not None and si.on_wait and len(si.on_wait) > 1
                        and not isinstance(inst, mybir.InstNoOp)):
                    waits = list(si.on_wait)
                    for i, w in enumerate(waits[:-1]):
                        new_insts.append(mybir.InstNoOp(
                            name=f"{inst.name}-wsplit{i}", engine=inst.engine,
                            bass_nofuse=True,
                            sync_info=mybir.SyncInfo(on_wait=[w], on_update=[])))
                        n += 1
                    inst.sync_info = mybir.SyncInfo(
                        on_wait=[waits[-1]], on_update=si.on_update)
                new_insts.append(inst)
            block.instructions[:] = new_insts
    return n


def _attention_chunk_pair(nc, C, hpair, khT_sb, qhT_sb, vh_sb, tri_sb, pt_pool,
                          st_pool, cx_pool, rec_pool, ctxT_sb):
    """Attention for sq-chunk C for head pair (2*hpair, 2*hpair+1). The two
    heads' K=64 S.T matmuls are emitted back-to-back targeting PE row groups
    0-63 / 64-127 so they run concurrently in the array."""
    mdt = _mmdt()
    heads = (2 * hpair, 2 * hpair + 1)
    cxs = {}
    for h in heads:
        cx_h = cx_pool.tile([P, QC], f32, tag="cx", name=f"cx{h}")
        cxs[h] = cx_h
    jmax = 4 * C + 3

    def st_stage(j):
        off = max(0, (j - 4 * C) * P)      # masked prefix width inside chunk
        lo = C * QC + off
        w = QC - off
        # pair-fused S.T psum / P.T tiles: head h at column block 512*(h%2)
        stp = st_pool.tile([P, 2 * QC], f32, tag="st")
        pt = pt_pool.tile([P, 2 * QC], mdt, tag="pt")
        for h in heads:
            pb = DH * (h % 2)
            kh = khT_sb[pb:pb + DH, h // 2, :]
            qh = qhT_sb[pb:pb + DH, h // 2, :]
            # S.T tile: [sk=128 of tile j, sq=w]; the two heads target PE row
            # groups 0-63 / 64-127 and run concurrently in the array.
            nc.tensor.matmul(stp[:, ds(QC * (h % 2), w)], kh[:, ts(j, P)],
                             qh[:, ds(lo, w)], start=True, stop=True)
        # one wide exp for both heads: P.T = exp(S.T / 8), PSUM -> SBUF
        pt_r = pt.rearrange("p (h q) -> p h q", h=2)
        stp_r = stp.rearrange("p (h q) -> p h q", h=2)
        nc.scalar.activation(
            out=pt_r[:, :, off:QC], in_=stp_r[:, :, 0:w],
            func=AF.Exp, scale=float(DH) ** -0.5)
        if j >= 4 * C:
            # diagonal tile: zero future positions (sq < sk); one fused op
            # over both heads with tri broadcast along the pair dim
            tri_bc = tri_sb.rearrange("p (o f) -> p o f", o=1).to_broadcast(
                [P, 2, P])
            nc.vector.tensor_tensor(
                out=pt_r[:, :, off:off + P], in0=pt_r[:, :, off:off + P],
                in1=tri_bc, op=OP.mult)
        return pt

    def pv_stage(j, pt):
        off = max(0, (j - 4 * C) * P)
        w = QC - off
        for h in heads:
            # PV accumulate: rows 0:64 ctx.T, rows 64:128 l (V ones-block).
            # The masked prefix [0, off) is skipped: those sq positions get
            # no contribution from sk-tile j.
            nc.tensor.matmul(cxs[h][:, off:QC],
                             vh_sb[:, j, h, :],
                             pt[:, ds(QC * (h % 2) + off, w)],
                             start=(j == 0), stop=(j == jmax),
                             skip_group_check=True)

    # 1-step software skew: each PV trails its S.T by one j so a PV stalled
    # on a cx slot always has the next S.T already issued on the PE
    pts = {}
    SKEW = 3
    for j in range(jmax + 1):
        pts[j] = st_stage(j)
        if j >= SKEW:
            pv_stage(j - SKEW, pts.pop(j - SKEW))
    for j in range(max(0, jmax - SKEW + 1), jmax + 1):
        pv_stage(j, pts.pop(j))
    for h in heads:
        # normalize: 1/l = exp(-ln(l)) on ACT (DVE reciprocal is ~6 cyc/elem
        # and its latency would sit on the PSUM-release chain); l is broadcast
        # in rows 64:128 by the V ones-block.
        cx = cxs[h]
        lt = rec_pool.tile([DH, QC], f32, tag="lt")
        nc.scalar.activation(out=lt[:, :], in_=cx[DH:2 * DH, :], func=AF.Ln)
        rinv = rec_pool.tile([DH, QC], f32, tag="rinv")
        nc.scalar.activation(out=rinv[:, :], in_=lt[:, :], func=AF.Exp,
                             scale=-1.0)
        pbase, pair = DH * (h % 2), h // 2
        nc.vector.tensor_tensor(
            out=ctxT_sb[pbase:pbase + DH, pair, ds(C * QC, QC)],
            in0=cx[0:DH, :], in1=rinv[:, :], op=OP.mult)


def build():
    mdt = _mmdt()
    SCH = _sch()
    NCH = S // SCH
    nc = bass.Bass("TRN2", enable_partition_id=False)

    qt = nc.dram_tensor("qt", [NCH, P, DT, SCH], mdt, kind="ExternalInput")
    kt = nc.dram_tensor("kt", [NCH, P, DT, SCH], mdt, kind="ExternalInput")
    vt = nc.dram_tensor("vt", [NCH, P, DT, SCH], mdt, kind="ExternalInput")
    wq = nc.dram_tensor("wq", [P, DT, E], mdt, kind="ExternalInput")
    wk = nc.dram_tensor("wk", [P, DT, E], mdt, kind="ExternalInput")
    wv = nc.dram_tensor("wv", [P, DT, E], mdt, kind="ExternalInput")
    wp = nc.dram_tensor("wp", [P, 2, D], mdt, kind="ExternalInput")
    bq2 = nc.dram_tensor("bq2", [P, ET], f32, kind="ExternalInput")
    bk2 = nc.dram_tensor("bk2", [P, ET], f32, kind="ExternalInput")
    bvr = nc.dram_tensor("bvr", [1, E], mdt, kind="ExternalInput")
    onesr = nc.dram_tensor("onesr", [1, E], mdt, kind="ExternalInput")
    tri = nc.dram_tensor("tri", [P, P], mdt, kind="ExternalInput")
    bpr = nc.dram_tensor("bpr", [1, D], mdt, kind="ExternalInput")
    outp = nc.dram_tensor("outp", [S, D], f32, kind="ExternalOutput")

    with tile.TileContext(nc) as tc:
        with (
            tc.tile_pool(name="consts", bufs=1) as consts,
            tc.tile_pool(name="inp", bufs=3) as inp,
            tc.tile_pool(name="acts", bufs=1) as acts,
            tc.tile_pool(name="pt", bufs=6) as pt_pool,
            tc.tile_pool(name="rec", bufs=4) as rec_pool,
            tc.tile_pool(name="osb", bufs=3) as osb_pool,
            tc.tile_pool(name="projps", bufs=2, space="PSUM") as proj_ps,
            tc.tile_pool(name="stps", bufs=2, space="PSUM") as st_pool,
            tc.tile_pool(name="cxps", bufs=2, space="PSUM") as cx_pool,
        ):
            out_ps = cx_pool  # out-proj psums share the cx slots (same shape)
            # ---- constants ----
            wq_sb = consts.tile([P, DT, E], mdt)
            wk_sb = consts.tile([P, DT, E], mdt)
            wv_sb = consts.tile([P, DT, E], mdt)
            wp_sb = consts.tile([P, 2, D], mdt)
            bq_sb = consts.tile([P, ET], f32)
            bk_sb = consts.tile([P, ET], f32)
            bvr_sb = consts.tile([1, E], mdt)
            ones_sb = consts.tile([1, E], mdt)
            tri_sb = consts.tile([P, P], mdt)
            bp_sb = consts.tile([P, D], mdt)


            # PE warmup: dummy matmuls during the startup DMA window release
            # the HAM clock throttle before the first real matmul arrives
            warm = consts.tile([P, QC], mdt)
            nc.vector.memset(warm[:].bitcast(
                mybir.dt.uint16 if MM_DTYPE == "bf16" else mybir.dt.uint32), 0)
            wps = proj_ps.tile([P, SCH], f32, tag="proj")
            for _ in range(26):
                nc.tensor.matmul(wps[:, :], warm[:, 0:P], warm[:, :],
                                 start=True, stop=True)

            # ---- persistent activations ----
            qhT_sb = acts.tile([P, ET, S], mdt)    # Qh.T  (e on partitions)
            khT_sb = acts.tile([P, ET, S], mdt)    # Kh.T
            vh_sb = acts.tile([P, NST, HC, 2 * DH], mdt)  # [V | ones-block]
            ctxT_sb = acts.tile([P, 2, S], mdt)    # normalized ctx.T head pairs

            # ones block of vh (cols 64:128) - broadcasts l in the PV matmul
            _one_fill(nc, vh_sb[:, :, :, DH:2 * DH])

            # attention/out-proj constants: needed later, DMA'd after chunk 0
            def late_const_dmas():
                nc.sync.dma_start(out=tri_sb[:], in_=tri[:, :])
                nc.sync.dma_start(out=wp_sb[:], in_=wp[:, :, :])
                nc.sync.dma_start(out=bp_sb[:],
                                  in_=bpr[0:1, :].to_broadcast([P, D]))

            helpers = {}
            for ch in range(NCH):
                qc = inp.tile([P, DT, SCH], mdt, tag="qc")
                kc = inp.tile([P, DT, SCH], mdt, tag="kc")
                vc = inp.tile([P, DT, SCH], mdt, tag="vc")
                if ch == 0:
                    # SP issues DMA triggers in order: sequence so the first
                    # Q-proj matmuls' deps (wq, bq, first half of qc) land
                    # first, then K's, then the rest.
                    hd = DT // 2
                    nc.sync.dma_start(out=wq_sb[:], in_=wq[:, :, :])
                    nc.sync.dma_start(out=bq_sb[:], in_=bq2[:, :])
                    nc.sync.dma_start(out=qc[:, 0:hd, :], in_=qt[ch, :, 0:hd, :])
                    nc.sync.dma_start(out=wk_sb[:], in_=wk[:, :, :])
                    nc.sync.dma_start(out=bk_sb[:], in_=bk2[:, :])
                    nc.sync.dma_start(out=kc[:, 0:hd, :], in_=kt[ch, :, 0:hd, :])
                    nc.sync.dma_start(out=qc[:, hd:DT, :],
                                      in_=qt[ch, :, hd:DT, :])
                    nc.sync.dma_start(out=kc[:, hd:DT, :],
                                      in_=kt[ch, :, hd:DT, :])
                    nc.sync.dma_start(out=wv_sb[:], in_=wv[:, :, :])
                    nc.sync.dma_start(out=bvr_sb[:], in_=bvr[:, :])
                    nc.sync.dma_start(out=ones_sb[:], in_=onesr[:, :])
                    nc.sync.dma_start(out=vc[:], in_=vt[ch])
                    late_const_dmas()
                else:
                    nc.sync.dma_start(out=qc[:], in_=qt[ch])
                    nc.sync.dma_start(out=kc[:], in_=kt[ch])
                    nc.sync.dma_start(out=vc[:], in_=vt[ch])

                # Q / K projections -> [e, s] layout
                for (src, w_sb, b_sb, dst) in (
                    (qc, wq_sb, bq_sb, qhT_sb),
                    (kc, wk_sb, bk_sb, khT_sb),
                ):
                    for et in range(ET):
                        pp = proj_ps.tile([P, SCH], f32, tag="proj")
                        for d in range(DT):
                            nc.tensor.matmul(pp[:, :], w_sb[:, d, ts(et, P)],
                                             src[:, d, :],
                                             start=(d == 0), stop=(d == DT - 1))
                        nc.vector.tensor_scalar_add(
                            out=dst[:, et, ts(ch, SCH)], in0=pp[:, :],
                            scalar1=b_sb[:, et:et + 1])

                # V projection -> [s, e] natural layout (+ bias via ones row)
                for st2 in range(SCH // P):
                    s_tile = ch * (SCH // P) + st2
                    pp = proj_ps.tile([P, SCH], f32, tag="proj")
                    for d in range(DT):
                        nc.tensor.matmul(pp[:, 0:E], vc[:, d, ds(st2 * P, P)],
                                         wv_sb[:, d, :],
                                         start=(d == 0), stop=False)
                    nc.tensor.matmul(pp[:, 0:E], ones_sb[0:1, 0:P],
                                     bvr_sb[0:1, :], start=False, stop=True)
                    # one fused strided copy scatters all 4 heads' V slices
                    nc.vector.tensor_copy(
                        out=vh_sb[:, s_tile, :, 0:DH],
                        in_=pp[:, 0:E].rearrange("p (h d) -> p h d", h=HC))

                def outproj(C):
                    split_dma = (C == NQC - 1)  # drain the tail sooner
                    for t in range(4 * C, 4 * C + 4):
                        o_sb = osb_pool.tile([P, D], f32, tag="osb")
                        for nch2 in range(2):
                            op = out_ps.tile([P, QC], f32, tag="cx")
                            for pair in range(2):
                                nc.tensor.matmul(
                                    op[:, :], ctxT_sb[:, pair, ts(t, P)],
                                    wp_sb[:, pair, ds(nch2 * QC, QC)],
                                    start=(pair == 0), stop=(pair == 1))
                            nc.vector.tensor_tensor(
                                out=o_sb[:, ds(nch2 * QC, QC)], in0=op[:, :],
                                in1=bp_sb[:, ds(nch2 * QC, QC)], op=OP.add)
                            if split_dma:
                                nc.sync.dma_start(
                                    out=outp[ts(t, P), ds(nch2 * QC, QC)],
                                    in_=o_sb[:, ds(nch2 * QC, QC)])
                        if not split_dma:
                            nc.sync.dma_start(out=outp[ts(t, P), :],
                                              in_=o_sb[:])

                def attn(C):
                    for hpair in range(HC // 2):
                        _attention_chunk_pair(nc, C, hpair, khT_sb, qhT_sb,
                                              vh_sb, tri_sb, pt_pool, st_pool,
                                              cx_pool, rec_pool, ctxT_sb)

                if ((ch + 1) * SCH) % QC == 0:
                    C = ((ch + 1) * SCH) // QC - 1
                    # previous chunk's out-proj here: its ctxT (normalize
                    # chain) is ready by now, so it doesn't stall the PE FIFO
                    # the way it would right after the attention pairs
                    if C > 0:
                        outproj(C - 1)
                    attn(C)
                    if C == NQC - 1:
                        outproj(C)
    legalize_multi_waits(nc)
    return nc


_NC_CACHE = {}


def _shard(inputs):
    q, k, v = inputs["q"], inputs["k"], inputs["v"]
    Wq, bq, Wk, bk = inputs["Wq"], inputs["bq"], inputs["Wk"], inputs["bk"]
    Wv, bv, Wp, bp = inputs["Wv"], inputs["bv"], inputs["Wp"], inputs["bp"]

    SCH = _sch()
    NCH = S // SCH

    def xT_layout(x):       # [S, D] -> [NCH, P, DT, SCH]
        a = _np_round(np.ascontiguousarray(np.asarray(x).T))      # [D, S]
        return np.ascontiguousarray(
            a.reshape(DT, P, NCH, SCH).transpose(2, 1, 0, 3))

    def w_layout(W, g):     # -> [P, DT, E]   (W[gdims,:].T tiled)
        a = _np_round(np.ascontiguousarray(np.asarray(W)[g * E:(g + 1) * E, :].T))
        return np.ascontiguousarray(a.reshape(DT, P, E).transpose(1, 0, 2))

    def wp_layout(Wp_, g):  # -> [P, 2, D]    (Wp[:, gdims].T tiled)
        a = _np_round(np.ascontiguousarray(np.asarray(Wp_)[:, g * E:(g + 1) * E].T))
        return np.ascontiguousarray(a.reshape(2, P, D).transpose(1, 0, 2))

    tri_np = _np_round(np.triu(np.ones((P, P), np.float32)))  # keep sq >= sk
    ones_np = _np_round(np.ones((1, E), np.float32))
    zeros_bp = np.zeros(D, np.float32)

    in_maps = []
    for c in range(8):
        b, g = c // 4, c % 4
        m = {
            "qt": xT_layout(q[b]),
            "kt": xT_layout(k[b]),
            "vt": xT_layout(v[b]),
            "wq": w_layout(Wq, g),
            "wk": w_layout(Wk, g),
            "wv": w_layout(Wv, g),
            "wp": wp_layout(Wp, g),
            "bq2": np.ascontiguousarray(
                np.asarray(bq, np.float32)[g * E:(g + 1) * E].reshape(ET, P).T),
            "bk2": np.ascontiguousarray(
                np.asarray(bk, np.float32)[g * E:(g + 1) * E].reshape(ET, P).T),
            "bvr": _np_round(np.asarray(bv)[g * E:(g + 1) * E].reshape(1, E)),
            "onesr": ones_np,
            "tri": tri_np,
            "bpr": _np_round(np.asarray(bp, np.float32).reshape(1, D))
                   if g == 0 else _np_round(zeros_bp.reshape(1, D)),
        }
        in_maps.append(m)
    return in_maps


def run(inputs, trace: bool = False, trace_cores=None):
    if MM_DTYPE not in _NC_CACHE:
        _NC_CACHE[MM_DTYPE] = build()
    nc = _NC_CACHE[MM_DTYPE]
    in_maps = _shard(inputs)
    kw = {}
    if trace:
        kw = dict(trace=True,
                  trace_cores=trace_cores if trace_cores is not None else [0])
    res = run_bass_kernel_spmd(nc, in_maps, core_ids=list(range(8)), **kw)
    out = np.zeros((B, S, D), np.float32)
    for c in range(8):
        out[c // 4] += res.results[c]["outp"]
    return out, res


def kernel(**inputs) -> np.ndarray:
    return run(inputs)[0]


# revision 67
# speedup vs baseline: 1.1173x; 1.0004x over previous
"""Causal multi-head attention (B=2, S=2048, D=1024, H=16) on 8 TRN2 NeuronCores.

Sharding: core c -> batch b = c//4, head-group g = c%4 (4 heads = 256 e-dims).
Each core computes its heads' attention plus the row-parallel slice of the out
projection; the host sums the 4 partial outputs per batch.

Matmul operand dtype is switchable (MM_DTYPE): bfloat16 (full PE rate) or
float32r (TF32-like, ~half rate, 8x less rounding error). Host pre-rounds
operands bitwise so device rounding is the identity. Scores are computed
transposed (S.T[sk, sq]) so softmax needs no reductions: exp without
max-subtraction (|scaled scores| < ~4), causal handled by a triangular mask
multiply after exp. V carries 64 extra all-ones columns so the PV matmul
produces both ctx.T (rows 0:64) and the softmax denominator broadcast to rows
64:128 for free; normalization is then reciprocal + multiply on DVE.
"""
import numpy as np
import ml_dtypes

import concourse.bass as bass
import concourse.tile as tile
from concourse import mybir
from concourse.bass import ts, ds
from concourse.bass_utils import run_bass_kernel_spmd

B, S, D, H = 2, 2048, 1024, 16
DH = D // H
P = 128
HC = 4            # heads per core
E = HC * DH       # 256 e-dims per core
ET = E // P       # 2 e-tiles
DT = D // P       # 8 d-tiles
NST = S // P      # 16 s-tiles
QC = 512          # attention sq-chunk
NQC = S // QC     # 4 attention chunks


def _sch():
    """Projection s-chunk width: 512 for bf16; 256 for f32r (SBUF budget)."""
    return 512 if MM_DTYPE == "bf16" else 256

f32 = mybir.dt.float32
AF = mybir.ActivationFunctionType
OP = mybir.AluOpType

MM_DTYPE = "bf16"   # "bf16" | "f32r"


def _mmdt():
    return mybir.dt.bfloat16 if MM_DTYPE == "bf16" else mybir.dt.float32r


def _np_round(x: np.ndarray) -> np.ndarray:
    """Host-side rounding matching the device matmul operand dtype."""
    x = np.ascontiguousarray(x, dtype=np.float32)
    if MM_DTYPE == "bf16":
        return x.astype(ml_dtypes.bfloat16)
    u = x.view(np.uint32).astype(np.uint64)
    u = u + 0x7FF + ((u >> 12) & 1)
    return (u & 0xFFFFF000).astype(np.uint32).view(np.float32)


def _zero(nc, ap):
    it = mybir.dt.uint16 if MM_DTYPE == "bf16" else mybir.dt.uint32
    nc.vector.memset(ap.bitcast(it), 0)


def _one_fill(nc, ap):
    if MM_DTYPE == "bf16":
        nc.vector.memset(ap.bitcast(mybir.dt.uint16), 0x3F80)
    else:
        nc.vector.memset(ap.bitcast(mybir.dt.uint32), 0x3F800000)


def legalize_multi_waits(nc) -> int:
    """Walrus STRUCT-decoded instructions accept a single sem wait; hoist extra
    waits onto preceding same-engine NoOps (same-engine program order preserves
    semantics exactly)."""
    n = 0
    for fn in nc.m.functions:
        for block in fn.blocks:
            new_insts = []
            for inst in block.instructions:
                si = getattr(inst, "sync_info", None)
                if (si is not None and si.on_wait and len(si.on_wait) > 1
                        and not isinstance(inst, mybir.InstNoOp)):
                    waits = list(si.on_wait)
                    for i, w in enumerate(waits[:-1]):
                        new_insts.append(mybir.InstNoOp(
                            name=f"{inst.name}-wsplit{i}", engine=inst.engine,
                            bass_nofuse=True,
                            sync_info=mybir.SyncInfo(on_wait=[w], on_update=[])))
                        n += 1
                    inst.sync_info = mybir.SyncInfo(
                        on_wait=[waits[-1]], on_update=si.on_update)
                new_insts.append(inst)
            block.instructions[:] = new_insts
    return n


def _attention_chunk_pair(nc, C, hpair, khT_sb, qhT_sb, vh_sb, tri_sb, pt_pool,
                          st_pool, cx_pool, rec_pool, ctxT_sb):
    """Attention for sq-chunk C for head pair (2*hpair, 2*hpair+1). The two
    heads' K=64 S.T matmuls are emitted back-to-back targeting PE row groups
    0-63 / 64-127 so they run concurrently in the array."""
    mdt = _mmdt()
    heads = (2 * hpair, 2 * hpair + 1)
    cxs = {}
    for h in heads:
        cx_h = cx_pool.tile([P, QC], f32, tag="cx", name=f"cx{h}")
        cxs[h] = cx_h
    jmax = 4 * C + 3

    def st_stage(j):
        off = max(0, (j - 4 * C) * P)      # masked prefix width inside chunk
        lo = C * QC + off
        w = QC - off
        # pair-fused S.T psum / P.T tiles: head h at column block 512*(h%2)
        stp = st_pool.tile([P, 2 * QC], f32, tag="st")
        pt = pt_pool.tile([P, 2 * QC], mdt, tag="pt")
        for h in heads:
            pb = DH * (h % 2)
            kh = khT_sb[pb:pb + DH, h // 2, :]
            qh = qhT_sb[pb:pb + DH, h // 2, :]
            # S.T tile: [sk=128 of tile j, sq=w]; the two heads target PE row
            # groups 0-63 / 64-127 and run concurrently in the array.
            nc.tensor.matmul(stp[:, ds(QC * (h % 2), w)], kh[:, ts(j, P)],
                             qh[:, ds(lo, w)], start=True, stop=True)
        # one wide exp for both heads: P.T = exp(S.T / 8), PSUM -> SBUF
        pt_r = pt.rearrange("p (h q) -> p h q", h=2)
        stp_r = stp.rearrange("p (h q) -> p h q", h=2)
        nc.scalar.activation(
            out=pt_r[:, :, off:QC], in_=stp_r[:, :, 0:w],
            func=AF.Exp, scale=float(DH) ** -0.5)
        if j >= 4 * C:
            # diagonal tile: zero future positions (sq < sk); one fused op
            # over both heads with tri broadcast along the pair dim
            tri_bc = tri_sb.rearrange("p (o f) -> p o f", o=1).to_broadcast(
                [P, 2, P])
            nc.vector.tensor_tensor(
                out=pt_r[:, :, off:off + P], in0=pt_r[:, :, off:off + P],
                in1=tri_bc, op=OP.mult)
        return pt

    def pv_stage(j, pt):
        off = max(0, (j - 4 * C) * P)
        w = QC - off
        for h in heads:
            # PV accumulate: rows 0:64 ctx.T, rows 64:128 l (V ones-block).
            # The masked prefix [0, off) is skipped: those sq positions get
            # no contribution from sk-tile j.
            nc.tensor.matmul(cxs[h][:, off:QC],
                             vh_sb[:, j, h, :],
                             pt[:, ds(QC * (h % 2) + off, w)],
                             start=(j == 0), stop=(j == jmax),
                             skip_group_check=True)

    # 1-step software skew: each PV trails its S.T by one j so a PV stalled
    # on a cx slot always has the next S.T already issued on the PE
    pts = {}
    SKEW = 4
    for j in range(jmax + 1):
        pts[j] = st_stage(j)
        if j >= SKEW:
            pv_stage(j - SKEW, pts.pop(j - SKEW))
    for j in range(max(0, jmax - SKEW + 1), jmax + 1):
        pv_stage(j, pts.pop(j))
    for h in heads:
        # normalize: 1/l = exp(-ln(l)) on ACT (DVE reciprocal is ~6 cyc/elem
        # and its latency would sit on the PSUM-release chain); l is broadcast
        # in rows 64:128 by the V ones-block.
        cx = cxs[h]
        lt = rec_pool.tile([DH, QC], f32, tag="lt")
        nc.scalar.activation(out=lt[:, :], in_=cx[DH:2 * DH, :], func=AF.Ln)
        rinv = rec_pool.tile([DH, QC], f32, tag="rinv")
        nc.scalar.activation(out=rinv[:, :], in_=lt[:, :], func=AF.Exp,
                             scale=-1.0)
        pbase, pair = DH * (h % 2), h // 2
        nc.vector.tensor_tensor(
            out=ctxT_sb[pbase:pbase + DH, pair, ds(C * QC, QC)],
            in0=cx[0:DH, :], in1=rinv[:, :], op=OP.mult)


def build():
    mdt = _mmdt()
    SCH = _sch()
    NCH = S // SCH
    nc = bass.Bass("TRN2", enable_partition_id=False)

    qt = nc.dram_tensor("qt", [NCH, P, DT, SCH], mdt, kind="ExternalInput")
    kt = nc.dram_tensor("kt", [NCH, P, DT, SCH], mdt, kind="ExternalInput")
    vt = nc.dram_tensor("vt", [NCH, P, DT, SCH], mdt, kind="ExternalInput")
    wq = nc.dram_tensor("wq", [P, DT, E], mdt, kind="ExternalInput")
    wk = nc.dram_tensor("wk", [P, DT, E], mdt, kind="ExternalInput")
    wv = nc.dram_tensor("wv", [P, DT, E], mdt, kind="ExternalInput")
    wp = nc.dram_tensor("wp", [P, 2, D], mdt, kind="ExternalInput")
    bq2 = nc.dram_tensor("bq2", [P, ET], f32, kind="ExternalInput")
    bk2 = nc.dram_tensor("bk2", [P, ET], f32, kind="ExternalInput")
    bvr = nc.dram_tensor("bvr", [1, E], mdt, kind="ExternalInput")
    onesr = nc.dram_tensor("onesr", [1, E], mdt, kind="ExternalInput")
    tri = nc.dram_tensor("tri", [P, P], mdt, kind="ExternalInput")
    bpr = nc.dram_tensor("bpr", [1, D], mdt, kind="ExternalInput")
    outp = nc.dram_tensor("outp", [S, D], f32, kind="ExternalOutput")

    with tile.TileContext(nc) as tc:
        with (
            tc.tile_pool(name="consts", bufs=1) as consts,
            tc.tile_pool(name="inp", bufs=3) as inp,
            tc.tile_pool(name="acts", bufs=1) as acts,
            tc.tile_pool(name="pt", bufs=8) as pt_pool,
            tc.tile_pool(name="rec", bufs=4) as rec_pool,
            tc.tile_pool(name="osb", bufs=3) as osb_pool,
            tc.tile_pool(name="projps", bufs=2, space="PSUM") as proj_ps,
            tc.tile_pool(name="stps", bufs=2, space="PSUM") as st_pool,
            tc.tile_pool(name="cxps", bufs=2, space="PSUM") as cx_pool,
        ):
            out_ps = cx_pool  # out-proj psums share the cx slots (same shape)
            # ---- constants ----
            wq_sb = consts.tile([P, DT, E], mdt)
            wk_sb = consts.tile([P, DT, E], mdt)
            wv_sb = consts.tile([P, DT, E], mdt)
            wp_sb = consts.tile([P, 2, D], mdt)
            bq_sb = consts.tile([P, ET], f32)
            bk_sb = consts.tile([P, ET], f32)
            bvr_sb = consts.tile([1, E], mdt)
            ones_sb = consts.tile([1, E], mdt)
            tri_sb = consts.tile([P, P], mdt)
            bp_sb = consts.tile([P, D], mdt)


            # PE warmup: dummy matmuls during the startup DMA window release
            # the HAM clock throttle before the first real matmul arrives
            warm = consts.tile([P, QC], mdt)
            nc.vector.memset(warm[:].bitcast(
                mybir.dt.uint16 if MM_DTYPE == "bf16" else mybir.dt.uint32), 0)
            wps = proj_ps.tile([P, SCH], f32, tag="proj")
            for _ in range(26):
                nc.tensor.matmul(wps[:, :], warm[:, 0:P], warm[:, :],
                                 start=True, stop=True)

            # ---- persistent activations ----
            qhT_sb = acts.tile([P, ET, S], mdt)    # Qh.T  (e on partitions)
            khT_sb = acts.tile([P, ET, S], mdt)    # Kh.T
            vh_sb = acts.tile([P, NST, HC, 2 * DH], mdt)  # [V | ones-block]
            ctxT_sb = acts.tile([P, 2, S], mdt)    # normalized ctx.T head pairs

            # ones block of vh (cols 64:128) - broadcasts l in the PV matmul
            _one_fill(nc, vh_sb[:, :, :, DH:2 * DH])

            # attention/out-proj constants: needed later, DMA'd after chunk 0
            def late_const_dmas():
                nc.sync.dma_start(out=tri_sb[:], in_=tri[:, :])
                nc.sync.dma_start(out=wp_sb[:], in_=wp[:, :, :])
                nc.sync.dma_start(out=bp_sb[:],
                                  in_=bpr[0:1, :].to_broadcast([P, D]))

            helpers = {}
            for ch in range(NCH):
                qc = inp.tile([P, DT, SCH], mdt, tag="qc")
                kc = inp.tile([P, DT, SCH], mdt, tag="kc")
                vc = inp.tile([P, DT, SCH], mdt, tag="vc")
                if ch == 0:
                    # SP issues DMA triggers in order: sequence so the first
                    # Q-proj matmuls' deps (wq, bq, first half of qc) land
                    # first, then K's, then the rest.
                    hd = DT // 2
                    nc.sync.dma_start(out=wq_sb[:], in_=wq[:, :, :])
                    nc.sync.dma_start(out=bq_sb[:], in_=bq2[:, :])
                    nc.sync.dma_start(out=qc[:, 0:hd, :], in_=qt[ch, :, 0:hd, :])
                    nc.sync.dma_start(out=wk_sb[:], in_=wk[:, :, :])
                    nc.sync.dma_start(out=bk_sb[:], in_=bk2[:, :])
                    nc.sync.dma_start(out=kc[:, 0:hd, :], in_=kt[ch, :, 0:hd, :])
                    nc.sync.dma_start(out=qc[:, hd:DT, :],
                                      in_=qt[ch, :, hd:DT, :])
                    nc.sync.dma_start(out=kc[:, hd:DT, :],
                                      in_=kt[ch, :, hd:DT, :])
                    nc.sync.dma_start(out=wv_sb[:], in_=wv[:, :, :])
                    nc.sync.dma_start(out=bvr_sb[:], in_=bvr[:, :])
                    nc.sync.dma_start(out=ones_sb[:], in_=onesr[:, :])
                    nc.sync.dma_start(out=vc[:], in_=vt[ch])
                    late_const_dmas()
                else:
                    nc.sync.dma_start(out=qc[:], in_=qt[ch])
                    nc.sync.dma_start(out=kc[:], in_=kt[ch])
                    nc.sync.dma_start(out=vc[:], in_=vt[ch])

                # Q / K projections -> [e, s] layout
                for (src, w_sb, b_sb, dst) in (
                    (qc, wq_sb, bq_sb, qhT_sb),
                    (kc, wk_sb, bk_sb, khT_sb),
                ):
                    for et in range(ET):
                        pp = proj_ps.tile([P, SCH], f32, tag="proj")
                        for d in range(DT):
                            nc.tensor.matmul(pp[:, :], w_sb[:, d, ts(et, P)],
                                             src[:, d, :],
                                             start=(d == 0), stop=(d == DT - 1))
                        nc.vector.tensor_scalar_add(
                            out=dst[:, et, ts(ch, SCH)], in0=pp[:, :],
                            scalar1=b_sb[:, et:et + 1])

                # V projection -> [s, e] natural layout (+ bias via ones row)
                for st2 in range(SCH // P):
                    s_tile = ch * (SCH // P) + st2
                    pp = proj_ps.tile([P, SCH], f32, tag="proj")
                    for d in range(DT):
                        nc.tensor.matmul(pp[:, 0:E], vc[:, d, ds(st2 * P, P)],
                                         wv_sb[:, d, :],
                                         start=(d == 0), stop=False)
                    nc.tensor.matmul(pp[:, 0:E], ones_sb[0:1, 0:P],
                                     bvr_sb[0:1, :], start=False, stop=True)
                    # one fused strided copy scatters all 4 heads' V slices
                    nc.vector.tensor_copy(
                        out=vh_sb[:, s_tile, :, 0:DH],
                        in_=pp[:, 0:E].rearrange("p (h d) -> p h d", h=HC))

                def outproj(C):
                    split_dma = (C == NQC - 1)  # drain the tail sooner
                    for t in range(4 * C, 4 * C + 4):
                        o_sb = osb_pool.tile([P, D], f32, tag="osb")
                        for nch2 in range(2):
                            op = out_ps.tile([P, QC], f32, tag="cx")
                            for pair in range(2):
                                nc.tensor.matmul(
                                    op[:, :], ctxT_sb[:, pair, ts(t, P)],
                                    wp_sb[:, pair, ds(nch2 * QC, QC)],
                                    start=(pair == 0), stop=(pair == 1))
                            nc.vector.tensor_tensor(
                                out=o_sb[:, ds(nch2 * QC, QC)], in0=op[:, :],
                                in1=bp_sb[:, ds(nch2 * QC, QC)], op=OP.add)
                            if split_dma:
                                nc.sync.dma_start(
                                    out=outp[ts(t, P), ds(nch2 * QC, QC)],
                                    in_=o_sb[:, ds(nch2 * QC, QC)])
                        if not split_dma:
                            nc.sync.dma_start(out=outp[ts(t, P), :],
                                              in_=o_sb[:])

                def attn(C):
                    for hpair in range(HC // 2):
                        _attention_chunk_pair(nc, C, hpair, khT_sb, qhT_sb,
                                              vh_sb, tri_sb, pt_pool, st_pool,
                                              cx_pool, rec_pool, ctxT_sb)

                if ((ch + 1) * SCH) % QC == 0:
                    C = ((ch + 1) * SCH) // QC - 1
                    # previous chunk's out-proj here: its ctxT (normalize
                    # chain) is ready by now, so it doesn't stall the PE FIFO
                    # the way it would right after the attention pairs
                    if C > 0:
                        outproj(C - 1)
                    attn(C)
                    if C == NQC - 1:
                        outproj(C)
    legalize_multi_waits(nc)
    return nc


_NC_CACHE = {}


def _shard(inputs):
    q, k, v = inputs["q"], inputs["k"], inputs["v"]
    Wq, bq, Wk, bk = inputs["Wq"], inputs["bq"], inputs["Wk"], inputs["bk"]
    Wv, bv, Wp, bp = inputs["Wv"], inputs["bv"], inputs["Wp"], inputs["bp"]

    SCH = _sch()
    NCH = S // SCH

    def xT_layout(x):       # [S, D] -> [NCH, P, DT, SCH]
        a = _np_round(np.ascontiguousarray(np.asarray(x).T))      # [D, S]
        return np.ascontiguousarray(
            a.reshape(DT, P, NCH, SCH).transpose(2, 1, 0, 3))

    def w_layout(W, g):     # -> [P, DT, E]   (W[gdims,:].T tiled)
        a = _np_round(np.ascontiguousarray(np.asarray(W)[g * E:(g + 1) * E, :].T))
        return np.ascontiguousarray(a.reshape(DT, P, E).transpose(1, 0, 2))

    def wp_layout(Wp_, g):  # -> [P, 2, D]    (Wp[:, gdims].T tiled)
        a = _np_round(np.ascontiguousarray(np.asarray(Wp_)[:, g * E:(g + 1) * E].T))
        return np.ascontiguousarray(a.reshape(2, P, D).transpose(1, 0, 2))

    tri_np = _np_round(np.triu(np.ones((P, P), np.float32)))  # keep sq >= sk
    ones_np = _np_round(np.ones((1, E), np.float32))
    zeros_bp = np.zeros(D, np.float32)

    in_maps = []
    for c in range(8):
        b, g = c // 4, c % 4
        m = {
            "qt": xT_layout(q[b]),
            "kt": xT_layout(k[b]),
            "vt": xT_layout(v[b]),
            "wq": w_layout(Wq, g),
            "wk": w_layout(Wk, g),
            "wv": w_layout(Wv, g),
            "wp": wp_layout(Wp, g),
            "bq2": np.ascontiguousarray(
                np.asarray(bq, np.float32)[g * E:(g + 1) * E].reshape(ET, P).T),
            "bk2": np.ascontiguousarray(
                np.asarray(bk, np.float32)[g * E:(g + 1) * E].reshape(ET, P).T),
            "bvr": _np_round(np.asarray(bv)[g * E:(g + 1) * E].reshape(1, E)),
            "onesr": ones_np,
            "tri": tri_np,
            "bpr": _np_round(np.asarray(bp, np.float32).reshape(1, D))
                   if g == 0 else _np_round(zeros_bp.reshape(1, D)),
        }
        in_maps.append(m)
    return in_maps


def run(inputs, trace: bool = False, trace_cores=None):
    if MM_DTYPE not in _NC_CACHE:
        _NC_CACHE[MM_DTYPE] = build()
    nc = _NC_CACHE[MM_DTYPE]
    in_maps = _shard(inputs)
    kw = {}
    if trace:
        kw = dict(trace=True,
                  trace_cores=trace_cores if trace_cores is not None else [0])
    res = run_bass_kernel_spmd(nc, in_maps, core_ids=list(range(8)), **kw)
    out = np.zeros((B, S, D), np.float32)
    for c in range(8):
        out[c // 4] += res.results[c]["outp"]
    return out, res


def kernel(**inputs) -> np.ndarray:
    return run(inputs)[0]
